# revision 2
# baseline (speedup 1.0000x reference)
"""Trainium2 Bass kernel v2 for nn_EnhancedQuantumLayer.

Data parallel over 8 cores (64 samples each); per core two phase-shifted
half-batch pipelines of 32 samples ([128, 256]-f32 tiles).

Key points vs v1:
  - State is complex-interleaved bf16 (re, im) pairs viewed as one f32
    element, so each layout flip is ONE StreamTranspose per half.
  - Product state X built by PE matmuls (u-chunked, runtime stationaries
    from the doubling output) -- no DRAM bounce / repack DMAs.
  - Feature-map recursion runs on Pool (gpsimd) as a single-engine chain.
  - PE warm-up dummy matmuls ramp the tensor clock to 2.4 GHz before the
    real matmul stream starts.
  - Output gathered by 3 strided DMAs directly from SBUF.

Sample index b (0..63) = (h:1 | s1:1 | s0:1 | u:3).
Layout A: p = 64*s1 + 32*q4 + 16*s0 + h4 ; f = 32*u + l5
Layout B: p = 64*s1 + 32*q4 + l5        ; f = 32*u + 16*s0 + h4
"""

import numpy as np

N_QUBITS = 10
N_LAYERS = 4
FREQS = (1.0, 2.0, 4.0, 8.0, 16.0)
PI = float(np.pi)
B_TOTAL = 512
B_CORE = 64
N_CORES = 8
N_WARM = 12  # PE warm-up dummy matmuls

CZCNOT = np.array([[1, 0, 0, 0],
                   [0, 1, 0, 0],
                   [0, 0, 0, -1],
                   [0, 0, 1, 0]], dtype=np.complex128)


# ---------------------------------------------------------------- host math
def _rz(phi):
    return np.array([[np.exp(-0.5j * phi), 0], [0, np.exp(0.5j * phi)]],
                    dtype=np.complex128)


def _rx(th):
    c, s = np.cos(th / 2), np.sin(th / 2)
    return np.array([[c, -1j * s], [-1j * s, c]], dtype=np.complex128)


def _ry(th):
    c, s = np.cos(th / 2), np.sin(th / 2)
    return np.array([[c, -s], [s, c]], dtype=np.complex128)


def _kron_list(ms):
    out = ms[0]
    for m in ms[1:]:
        out = np.kron(out, m)
    return out


def _embed_2q(space_qubits, qa, qb, M4):
    n = len(space_qubits)
    dim = 2 ** n
    pa, pb = space_qubits.index(qa), space_qubits.index(qb)
    out = np.zeros((dim, dim), dtype=np.complex128)
    for idx in range(dim):
        bits = [(idx >> (n - 1 - i)) & 1 for i in range(n)]
        col4 = 2 * bits[pa] + bits[pb]
        for row4 in range(4):
            val = M4[row4, col4]
            if val != 0:
                nb = bits.copy()
                nb[pa], nb[pb] = row4 >> 1, row4 & 1
                ridx = sum(bit << (n - 1 - i) for i, bit in enumerate(nb))
                out[ridx, idx] += val
    return out


A6 = [4, "s0", 0, 1, 2, 3]
L6 = [4, 5, 6, 7, 8, 9]
_E_evenA6 = _embed_2q(A6, 0, 1, CZCNOT) @ _embed_2q(A6, 2, 3, CZCNOT)
_E_oddA6 = _embed_2q(A6, 3, 4, CZCNOT) @ _embed_2q(A6, 1, 2, CZCNOT)
_PermA6 = _E_oddA6 @ _E_evenA6
_E_evenL6 = _embed_2q(L6, 6, 7, CZCNOT) @ _embed_2q(L6, 8, 9, CZCNOT)
_E_oddL6 = _embed_2q(L6, 7, 8, CZCNOT) @ _embed_2q(L6, 5, 6, CZCNOT)
_CG64 = _embed_2q(L6, 4, 5, CZCNOT)


def _layer_matrices6(theta):
    ang = np.tanh(theta.astype(np.float64)) * PI
    S_A, S_L = [], []
    for layer in range(N_LAYERS):
        U = []
        for q in range(10):
            a0, a1, a2 = ang[layer, q]
            U.append(_rx(a0 * 0.5) @ _rz(a2) @ _ry(a1) @ _rz(a0))
        UA6 = _kron_list([U[4], np.eye(2), U[0], U[1], U[2], U[3]])
        UL6 = _kron_list([np.eye(2), U[5], U[6], U[7], U[8], U[9]])
        S_A.append(UA6 if layer == 0 else UA6 @ _PermA6)
        S_L.append(_E_oddL6 @ _E_evenL6 @ _CG64 @ UL6)
    return S_A, S_L


def _measurement_weights6():
    W1 = np.zeros((128, 32), dtype=np.float32)
    for s1 in range(2):
        for q4 in range(2):
            for l in range(32):
                p = 64 * s1 + 32 * q4 + l
                W1[p, 16 * s1 + 0] = 1.0 - 2.0 * q4
                for j in range(5):
                    W1[p, 16 * s1 + 1 + j] = 1.0 - 2.0 * ((l >> (4 - j)) & 1)
                W1[p, 16 * s1 + 6] = 1.0
    W2 = np.zeros((32, 16), dtype=np.float32)
    for s0 in range(2):
        for h4 in range(16):
            p = 16 * s0 + h4
            s = [1.0 - 2.0 * ((h4 >> (3 - i)) & 1) for i in range(4)]
            W2[p, 4 * s0 + 0] = s[0]
            W2[p, 4 * s0 + 1] = s[1] * s[0]
            W2[p, 4 * s0 + 2] = s[2] * s[1] * s[0]
            W2[p, 4 * s0 + 3] = s[3] * s[2]
            W2[p, 8 + s0] = 1.0
            W2[p, 10 + s0] = s[3] * s[2]
    return W1, W2


def _host_weights(theta):
    """wstack [128, 24*128] bf16-as-uint16: wstack[k, 128*m + c] = S_m[c, k].
    Order per layer: [Ar, Ain(-Ai), Ai, Lr, Lin, Li]."""
    S_A, S_L = _layer_matrices6(theta)
    I2 = np.eye(2)
    w = np.zeros((128, 24 * 128), np.float32)
    m = 0
    for layer in range(N_LAYERS):
        for S in [S_A[layer], S_L[layer]]:
            full = np.kron(I2, S)
            for comp in (full.real, -full.imag, full.imag):
                w[:, 128 * m:128 * m + 128] = comp.T.astype(np.float32)
                m += 1
    # f32 -> bf16 (round-to-nearest-even) stored as uint16
    u = w.view(np.uint32)
    rounded = ((u + 0x7FFF + ((u >> 16) & 1)) >> 16).astype(np.uint16)
    return np.ascontiguousarray(rounded)


def _f32_to_bf16_u16(a):
    a = np.ascontiguousarray(a.astype(np.float32))
    u = a.view(np.uint32)
    return (((u + 0x7FFF + ((u >> 16) & 1)) >> 16)).astype(np.uint16)


# ------------------------------------------------------------- bass builder
_BUILD_CACHE = {}


def _build_module():
    import concourse.bass as bass  # noqa
    import concourse.mybir as mybir
    from concourse import bacc
    from concourse.tile import TileContext

    f32 = mybir.dt.float32
    bf16 = mybir.dt.bfloat16
    AF = mybir.ActivationFunctionType
    OP = mybir.AluOpType

    nc = bacc.Bacc("TRN2", target_bir_lowering=False, debug=False)

    xin = nc.dram_tensor("xin", [B_CORE, 10], f32, kind="ExternalInput").ap()
    # bf16 payload shipped as uint16 (run_bass_kernel_spmd feeds numpy arrays)
    wstack = nc.dram_tensor("wstack", [128, 24 * 128], mybir.dt.uint16,
                            kind="ExternalInput").ap()
    out_d = nc.dram_tensor("out", [B_CORE, 10], f32, kind="ExternalOutput").ap()

    # ---- constants
    # trig planes: rows j=0..5 sin(c_j x), rows 6..11 cos(c_j x), c_j=0.25*2^j
    cs_mult = np.zeros((12, 10), dtype=np.float32)
    cs_bias = np.zeros((12, 10), dtype=np.float32)
    for j in range(6):
        cs_mult[j] = 0.25 * 2 ** j
        cs_mult[6 + j] = 0.25 * 2 ** j
        cs_bias[6 + j] = 0.5 * PI
    mult_np = np.tile(cs_mult.reshape(1, 120), (64, 1)).astype(np.float32)
    bias_np = np.tile(cs_bias.reshape(1, 120), (64, 1)).astype(np.float32)
    v0_np = np.zeros((64, 40), dtype=np.float32)
    v0_np[:, 0::4] = 1.0
    cpack = np.zeros((64, 280), dtype=np.float32)
    cpack[:, 0:120] = mult_np
    cpack[:, 120:240] = bias_np
    cpack[:, 240:280] = v0_np
    cpack_c = nc.inline_tensor(cpack, name="cpack").ap()

    W1_np, W2_np = _measurement_weights6()
    mask_np = np.zeros((64, 128), np.float32)
    for p in range(64):
        s1, s0 = (p >> 4) & 1, (p >> 3) & 1
        for q4 in range(2):
            c0 = 64 * s1 + 32 * q4 + 16 * s0
            mask_np[p, c0:c0 + 16] = 1.0
    mcmask_np = np.zeros((64, 512), np.float32)
    for p in range(64):
        u = p & 7
        mcmask_np[p, 64 * u:64 * u + 64] = 1.0
    wpack = np.zeros((128, 816), np.uint16)
    wpack[:, 0:32] = _f32_to_bf16_u16(W1_np)
    wpack[0:32, 32:48] = _f32_to_bf16_u16(W2_np)
    wpack[0:64, 48:176] = _f32_to_bf16_u16(mask_np)
    wpack[0:64, 176:304] = _f32_to_bf16_u16(-mask_np)
    wpack[0:64, 304:816] = _f32_to_bf16_u16(mcmask_np)
    wpack_c = nc.inline_tensor(wpack, name="wpack").ap()

    with TileContext(nc) as tc:
        with (
            tc.tile_pool(name="wpool", bufs=1) as wpool,
            tc.tile_pool(name="st", bufs=1) as st,      # state tiles
            tc.tile_pool(name="sm", bufs=2) as sm,      # small tiles
            tc.tile_pool(name="ps", bufs=1, space="PSUM") as ps,
        )            :
            # ---------------- phase 0: DMAs + PE warm-up
            # partition p = sample b = 32h + 16s1 + 8s0 + u
            sx = sm.tile([64, 10], f32, tag="sx")
            nc.sync.dma_start(sx[:], xin)
            ct = wpool.tile([64, 280], f32, tag="cp")
            nc.sync.dma_start(ct[:], cpack_c)
            wt = wpool.tile([128, 24 * 128], bf16, tag="w")
            nc.sync.dma_start(wt[:].bitcast(mybir.dt.uint16), wstack)
            mult_v = ct[:, 0:120]
            bias_v = ct[:, 120:240]
            v0_v = ct[:, 240:280]
            wq = wpool.tile([128, 816], bf16, tag="wq")
            w1_t = wq[:, 0:32]
            w2_t = wq[0:32, 32:48]
            mask_v = wq[0:64, 48:176]
            nmask_v = wq[0:64, 176:304]
            mcmask_v = wq[0:64, 304:816]

            junk = wpool.tile([128, 512], bf16, tag="junk")
            nc.gpsimd.memset(junk[:], 0.125)
            warm = ps.tile([128, 512], f32, tag="y0")
            for k in range(N_WARM):
                nc.tensor.matmul(warm[:], junk[:, 0:128], junk[:],
                                 start=True, stop=True)

            # ---------------- phase 1-2: tanh + trig table
            xt = sm.tile([64, 10], f32, tag="xt")
            nc.scalar.activation(xt[:], sx[:], AF.Tanh)
            nc.scalar.dma_start(wq[:].bitcast(mybir.dt.uint16), wpack_c)

            xb12 = (xt[:].unsqueeze(1).broadcast_to((64, 12, 10)))
            ma = sm.tile([64, 120], f32, tag="ma")
            nc.vector.tensor_tensor(
                ma[:].rearrange("p (r q) -> p r q", q=10), xb12,
                mult_v.rearrange("p (r q) -> p r q", q=10), OP.mult)
            nc.vector.tensor_tensor(ma[:], ma[:], bias_v, OP.add)
            MAGIC = 1.5 * 2 ** 23
            kk = sm.tile([64, 120], f32, tag="kk")
            nc.vector.tensor_scalar(kk[:], ma[:], 1.0 / (2.0 * PI), MAGIC,
                                    OP.mult, OP.add)
            nc.vector.tensor_scalar(kk[:], kk[:], MAGIC, None, OP.subtract)
            nc.vector.scalar_tensor_tensor(ma[:], kk[:], -2.0 * PI, ma[:],
                                           OP.mult, OP.add)
            PCLAMP = PI * (1.0 - 1e-6)
            nc.vector.tensor_scalar(ma[:], ma[:], PCLAMP, -PCLAMP,
                                    OP.min, OP.max)

            # tb rows: s (0:60) | ns (60:120) | s2 (120:180) | cos (180:240)
            tb = sm.tile([64, 240], f32, tag="tb")
            # chunk A: freqs 0-2 ; chunk B: freqs 3-5
            for lo, hi in ((0, 30), (30, 60)):
                nc.scalar.activation(tb[:, lo:hi], ma[:, lo:hi], AF.Sin)
                # cos rows
                nc.scalar.activation(tb[:, 180 + lo:180 + hi],
                                     ma[:, 60 + lo:60 + hi], AF.Sin)
                # ns = -s ; s2 = s
                nc.gpsimd.tensor_scalar(tb[:, 60 + lo:60 + hi],
                                        tb[:, lo:hi], -1.0, None, OP.mult)
                nc.gpsimd.tensor_copy(tb[:, 120 + lo:120 + hi], tb[:, lo:hi])

            tb_v = tb[:].rearrange("p (r q) -> p r q", q=10)  # [64, 24, 10]

            # ---------------- phase 3: v-recursion on Pool
            v_cur = None
            for k in range(10):
                is_rz = (k % 2 == 0)
                lv = k // 2 + 1 if is_rz else k // 2
                cplane = (tb_v[:, 18 + lv, :].unsqueeze(2)
                          .broadcast_to((64, 10, 4))
                          .rearrange("p q (a b) -> p q a b", a=2))
                t1 = sm.tile([64, 40], f32, tag="t1")
                t2 = sm.tile([64, 40], f32, tag="t2")
                t1v = t1[:].rearrange("p (q a b) -> p q a b", a=2, b=2)
                t2v = t2[:].rearrange("p (q a b) -> p q a b", a=2, b=2)
                vsrc = v0_v if v_cur is None else v_cur[:]
                vv = vsrc.rearrange("p (q a b) -> p q a b", a=2, b=2)
                nc.gpsimd.tensor_tensor(t1v, vv, cplane, OP.mult)
                spq = tb[:].rearrange("p (r q) -> p q r", q=10)  # [64,10,24]
                if is_rz:
                    # t2[t, c] = vpart[t, c] * row(lv + 6*(t+c)); (s, ns, s2)
                    vpart = vv[:, :, :, ::-1]
                    s_t0 = spq[:, :, lv:lv + 7:6].unsqueeze(2)    # [64,10,1,2]
                    s_t1 = spq[:, :, lv + 6:lv + 13:6].unsqueeze(2)
                    nc.gpsimd.tensor_tensor(t2v[:, :, 0:1, :],
                                            vpart[:, :, 0:1, :], s_t0, OP.mult)
                    nc.gpsimd.tensor_tensor(t2v[:, :, 1:2, :],
                                            vpart[:, :, 1:2, :], s_t1, OP.mult)
                else:
                    vpart = vv[:, :, ::-1, ::-1]
                    sview = (spq[:, :, lv:lv + 7:6].unsqueeze(2)
                             .broadcast_to((64, 10, 2, 2)))
                    nc.gpsimd.tensor_tensor(t2v, vpart, sview, OP.mult)
                v_nxt = sm.tile([64, 40], f32, tag="vb" if k % 2 == 0 else "va")
                nc.gpsimd.tensor_tensor(v_nxt[:], t1[:], t2[:], OP.add)
                v_cur = v_nxt

            # ---------------- phase 4: H/L doubling (Pool), G [64,64] r/i
            H_QUBITS = [4, 0, 1, 2, 3]
            L_QUBITS = [5, 6, 7, 8, 9]
            g_ar = sm.tile([64, 64], f32, tag="gra")
            g_ai = sm.tile([64, 64], f32, tag="gia")
            g_br = sm.tile([64, 64], f32, tag="grb")
            g_bi = sm.tile([64, 64], f32, tag="gib")
            g_r, g_i = g_ar, g_ai
            vvq = v_cur[:].rearrange("p (q t c) -> p q t c", t=2, c=2)
            g_r0 = g_r[:].rearrange("p (s x) -> p s x", s=2)[:, :, 0:2]
            g_i0 = g_i[:].rearrange("p (s x) -> p s x", s=2)[:, :, 0:2]
            nc.gpsimd.tensor_copy(g_r0, vvq[:, 4:6, :, 0])
            nc.gpsimd.tensor_copy(g_i0, vvq[:, 4:6, :, 1])
            ptA_full = sm.tile([64, 128], f32, tag="ptA")
            ptB_full = sm.tile([64, 128], f32, tag="ptB")
            for j in range(1, 5):
                w = 2 ** j
                qH = H_QUBITS[j]
                ptA = ptA_full[:, 0:8 * w]
                ptB = ptB_full[:, 0:8 * w]
                gr_b = (g_r[:].rearrange("p (s x) -> p s x", s=2)[:, :, 0:w]
                        .unsqueeze(3).broadcast_to((64, 2, w, 2)))
                gi_b = (g_i[:].rearrange("p (s x) -> p s x", s=2)[:, :, 0:w]
                        .unsqueeze(3).broadcast_to((64, 2, w, 2)))
                vsel = vvq[:, qH:qH + 7:6]          # [64, 2q, 2t, 2c]
                vA = (vsel.transpose([0, 3, 1, 2])
                      .unsqueeze(3).broadcast_to((64, 2, 2, w, 2)))
                vB = (vsel[:, :, :, ::-1].transpose([0, 3, 1, 2])
                      .unsqueeze(3).broadcast_to((64, 2, 2, w, 2)))
                ptA_v = ptA.rearrange("p (c s x t) -> p c s x t",
                                      c=2, s=2, t=2)
                ptB_v = ptB.rearrange("p (c s x t) -> p c s x t",
                                      c=2, s=2, t=2)
                for c in range(2):
                    nc.gpsimd.tensor_tensor(ptA_v[:, c], gr_b, vA[:, c],
                                            OP.mult)
                    nc.gpsimd.tensor_tensor(ptB_v[:, c], gi_b, vB[:, c],
                                            OP.mult)
                g2_r, g2_i = (g_br, g_bi) if j % 2 else (g_ar, g_ai)
                g2r_v = g2_r[:].rearrange("p (s h t) -> p s h t",
                                          s=2, t=2)[:, :, 0:w, :]
                g2i_v = g2_i[:].rearrange("p (s h t) -> p s h t",
                                          s=2, t=2)[:, :, 0:w, :]
                nc.gpsimd.tensor_tensor(g2r_v, ptA_v[:, 0], ptB_v[:, 0],
                                        OP.subtract)
                nc.gpsimd.tensor_tensor(g2i_v, ptA_v[:, 1], ptB_v[:, 1],
                                        OP.add)
                g_r, g_i = g2_r, g2_i

            # ---------------- phase 5: S_H assembly + M_c build
            # sh_* [64, 128] bf16: row p; cols = 64s1' + 32q4 + 16s0' + h4,
            # = G_H[p, (q4, h4)] * mask(s1'=s1(p), s0'=s0(p))
            sh_r = st.tile([64, 128], bf16, tag="shr")
            sh_i = st.tile([64, 128], bf16, tag="shi")
            sh_in = st.tile([64, 128], bf16, tag="shin")

            def bc(gt):
                return (gt[:, 0:32]
                        .rearrange("p (q4 h4) -> p q4 h4", q4=2)
                        .unsqueeze(2)
                        .broadcast_to((64, 2, 2, 16)))

            def shv(sh, a):
                return (sh[:, 64 * a:64 * a + 64]
                        .rearrange("p (q4 b h4) -> p q4 b h4", q4=2, b=2))

            def mkv(mk, a):
                return (mk[:, 64 * a:64 * a + 64]
                        .rearrange("p (q4 b h4) -> p q4 b h4", q4=2, b=2))

            for a in range(2):
                nc.gpsimd.tensor_tensor(shv(sh_r, a), bc(g_r),
                                        mkv(mask_v, a), OP.mult)
                nc.vector.tensor_tensor(shv(sh_i, a), bc(g_i),
                                        mkv(mask_v, a), OP.mult)
                nc.gpsimd.tensor_tensor(shv(sh_in, a), bc(g_i),
                                        mkv(nmask_v, a), OP.mult)
            # gl_c [64, 64] bf16 interleaved (Gr, Gi) pairs
            gl_c = st.tile([64, 64], bf16, tag="glc")
            glv = gl_c[:].rearrange("p (l c) -> p l c", c=2)
            nc.vector.tensor_copy(glv[:, :, 0], g_r[:, 32:64])
            nc.vector.tensor_copy(glv[:, :, 1], g_i[:, 32:64])
            # M_c [64, 512] bf16: row p -> cols 64u(p)..+64 = gl_c row p
            # dense broadcast * mask (diagonal not AP-expressible)
            m_c = st.tile([64, 512], bf16, tag="mc")
            nc.vector.tensor_tensor(
                m_c[:].rearrange("p (v x) -> p v x", v=8),
                gl_c[:].unsqueeze(1).broadcast_to((64, 8, 64)),
                mcmask_v.rearrange("p (v x) -> p v x", v=8), OP.mult)

            # ---------------- phase 6: X-build MMs (4 per half)
            def ri(bank):
                return bank[:, 0:256], bank[:, 256:512]

            def evict1(dst_f32tile, bank, h=0):
                # one op: [p, (c,f)] psum -> interleaved bf16 [p, (f,c)]
                dst = (dst_f32tile[:].bitcast(bf16)
                       .rearrange("p (f c) -> p c f", c=2))
                srcv = bank[:].rearrange("p (c f) -> p c f", c=2)
                nc.scalar.activation(dst, srcv, AF.Copy)

            x_bank = [ps.tile([128, 512], f32, tag=f"x{h}", name=f"xb{h}")
                      for h in range(2)]
            x_ps = [ri(t) for t in x_bank]
            for h in range(2):
                xr, xi = x_ps[h]
                rows = slice(32 * h, 32 * h + 32)
                mv = m_c[rows, :].rearrange("p (f c) -> p f c", c=2)
                m_ev, m_od = mv[:, :, 0], mv[:, :, 1]
                nc.tensor.matmul(xr, sh_r[rows, :], m_ev,
                                 start=True, stop=False)
                nc.tensor.matmul(xr, sh_in[rows, :], m_od,
                                 start=False, stop=True)
                nc.tensor.matmul(xi, sh_r[rows, :], m_od,
                                 start=True, stop=False)
                nc.tensor.matmul(xi, sh_i[rows, :], m_ev,
                                 start=False, stop=True)

            # ---------------- phase 7: layers
            def W(m):
                return wt[:, 128 * m:128 * m + 128]

            def evict(dst_f32tile, src_ps_r, src_ps_i, eng_r, eng_i):
                dv = dst_f32tile[:].bitcast(bf16).rearrange(
                    "p (f c) -> p f c", c=2)
                for eng, view, srcp in ((eng_r, dv[:, :, 0], src_ps_r),
                                        (eng_i, dv[:, :, 1], src_ps_i)):
                    if eng is nc.scalar:
                        nc.scalar.activation(view, srcp, AF.Copy)
                    else:
                        eng.tensor_copy(view, srcp)

            xa = [st.tile([128, 256], f32, tag=f"xa{h}", name=f"xa{h}")
                  for h in range(2)]
            xb = [st.tile([128, 256], f32, tag=f"xb{h}", name=f"xb{h}")
                  for h in range(2)]
            for h in range(2):
                evict1(xa[h], x_bank[h], h)

            def re_im(t):
                v = t[:].bitcast(bf16).rearrange("p (f c) -> p f c", c=2)
                return v[:, :, 0], v[:, :, 1]

            z_ps = [None, None]
            z_bank = [None, None]
            for layer in range(N_LAYERS):
                base = 6 * layer
                y_bank = [ps.tile([128, 512], f32, tag=f"y{h}",
                                  name=f"ybank{h}") for h in range(2)]
                y_ps = [ri(t) for t in y_bank]
                # A-side MMs (both halves)
                for h in range(2):
                    yr, yi = y_ps[h]
                    xre, xim = re_im(xa[h])
                    nc.tensor.matmul(yr, W(base + 0), xre,
                                     start=True, stop=False)
                    nc.tensor.matmul(yr, W(base + 1), xim,
                                     start=False, stop=True)
                    nc.tensor.matmul(yi, W(base + 0), xim,
                                     start=True, stop=False)
                    nc.tensor.matmul(yi, W(base + 2), xre,
                                     start=False, stop=True)
                # one full-bank evict (ACT) + one combined flip (DVE)
                for h in range(2):
                    yb = st.tile([128, 256], f32, tag=f"yb{h}")
                    evict1(yb, y_bank[h], h)
                    nc.vector.transpose(xb[h][:], yb[:])
                # L-side MMs
                for h in range(2):
                    zb_t = ps.tile([128, 512], f32, tag=f"z{h}",
                                   name=f"zbank{h}")
                    z_bank[h] = zb_t
                    z_ps[h] = ri(zb_t)
                    zr, zi = z_ps[h]
                    bre, bim = re_im(xb[h])
                    nc.tensor.matmul(zr, W(base + 3), bre,
                                     start=True, stop=False)
                    nc.tensor.matmul(zr, W(base + 4), bim,
                                     start=False, stop=True)
                    nc.tensor.matmul(zi, W(base + 3), bim,
                                     start=True, stop=False)
                    nc.tensor.matmul(zi, W(base + 5), bre,
                                     start=False, stop=True)
                if layer < N_LAYERS - 1:
                    for h in range(2):
                        zb = st.tile([128, 256], f32, tag=f"zb{h}")
                        evict1(zb, z_bank[h], h)
                        nc.vector.transpose(xa[h][:], zb[:])

            # ---------------- phase 8: measurement (layout B, per half)
            res = st.tile([32, 512], f32, tag="res")
            nc.gpsimd.memset(res[:], 0.0)
            res_t = st.tile([32, 512], f32, tag="rest")
            o_bank = ps.tile([32, 512], f32, tag="obank")
            o2_bank = ps.tile([16, 512], f32, tag="o2bank")
            for h in range(2):
                sq = st.tile([128, 512], f32, tag=f"sq{h}")
                nc.scalar.square(sq[:], z_bank[h][:])
                p_bf = st.tile([128, 256], bf16, tag=f"pbf{h}")
                nc.gpsimd.tensor_tensor(p_bf[:], sq[:, 0:256],
                                        sq[:, 256:512], OP.add)
                o1 = o_bank[:, 256 * h:256 * h + 256]
                nc.tensor.matmul(o1, w1_t, p_bf[:], start=True, stop=True)
                o1t = st.tile([32, 256], f32, tag=f"o1t{h}")
                nc.vector.transpose(o1t[:], o1)
                o1b = st.tile([32, 256], bf16, tag=f"o1b{h}")
                nc.gpsimd.tensor_copy(o1b[:], o1t[:])
                o2 = o2_bank[:, 256 * h:256 * h + 256]
                nc.tensor.matmul(o2, w2_t, o1b[:], start=True, stop=True)
                nc.scalar.activation(res[0:16, 256 * h:256 * h + 256],
                                     o2, AF.Copy)
                nc.vector.transpose(res_t[:, 256 * h:256 * h + 256],
                                    res[:, 256 * h:256 * h + 256])

            # ---------------- phase 9: gather
            # res[j2, 256h + 32u + 16s1 + j], j2 = 4 s0 + t (t<4) | 8+s0
            # out row r = 32h + 16 s0 + 2u + s1 (host un-permutes)
            # res_t[16 s1 + j, 256h + 32u + j2]
            out_v = out_d.rearrange("(h s0 u s1) q -> h s0 u s1 q",
                                    h=2, s0=2, u=8)
            res_v2 = res[:].rearrange("p (h u s1 j) -> p h u s1 j",
                                      h=2, u=8, s1=2)
            rtv = res_t[:].rearrange("p (h u f) -> p h u f", h=2, f=32)
            # piece 1 (q0-3): per (s1, s0): src row 16s1+6: (h, u, t)
            for s1 in range(2):
                for s0 in range(2):
                    eng = nc.sync if s0 == 0 else nc.scalar
                    eng.dma_start(
                        out_v[:, s0, :, s1, 0:4].opt(),
                        rtv[16 * s1 + 6:16 * s1 + 7, :, :,
                            4 * s0:4 * s0 + 4].opt())
            for h in range(2):
                # piece 2 (q4): rows 10:12 (dup W2 cols), j=0
                nc.gpsimd.dma_start(
                    out_v[h, :, :, :, 4:5],
                    res_v2[10:12, h, :, :, 0:1])
                # piece 3 (q5-9): rows 8:10
                eng = nc.sync if h == 0 else nc.scalar
                eng.dma_start(
                    out_v[h, :, :, :, 5:10],
                    res_v2[8:10, h, :, :, 1:6])

    import concourse.bacc as _bacc
    _orig_tables = _bacc.get_activation_tables

    def _patched_tables(arch):
        t = dict(_orig_tables(arch))
        AFt = mybir.ActivationFunctionType
        strip = {AFt.Tanh, AFt.Sin, AFt.Square, AFt.Copy}
        out = {}
        for name, s in t.items():
            out[name] = s if name == "silu_and_others" else (s - strip)
        return out

    _bacc.get_activation_tables = _patched_tables
    try:
        nc.finalize()
    finally:
        _bacc.get_activation_tables = _orig_tables
    # the pass emits an unconditional set-0 load at entry plus the real
    # set load before the first activation; merge them into one.
    for blk in nc.m.functions[0].blocks:
        insts = blk.instructions
        loads = [(i, x) for i, x in enumerate(insts)
                 if isinstance(x, mybir.InstLoadActFuncSet)]
        if (len(loads) == 2 and not any(x.has_wait() or x.has_update()
                                        for _, x in loads)):
            loads[0][1].act_func_set_id = loads[1][1].act_func_set_id
            del insts[loads[1][0]]
    return nc


def _get_module():
    if "nc" not in _BUILD_CACHE:
        _BUILD_CACHE["nc"] = _build_module()
    return _BUILD_CACHE["nc"]


# ---------------------------------------------------------------- entrypoint
def _out_perm():
    """outp[b] = device-out row holding sample b's result.
    Device writes sample (h,s1,s0,u) to row r = 32h + 16s0 + 2u + s1."""
    outp = np.zeros(64, np.int64)
    for b in range(64):
        h, s1, s0, u = b >> 5, (b >> 4) & 1, (b >> 3) & 1, b & 7
        outp[b] = 32 * h + 16 * s0 + 2 * u + s1
    return outp


def kernel(inputs, theta):
    inputs = np.asarray(inputs, dtype=np.float32)
    theta = np.asarray(theta, dtype=np.float32)
    assert inputs.shape == (B_TOTAL, N_QUBITS)

    from concourse.bass_utils import run_bass_kernel_spmd

    nc = _get_module()
    wstack = _host_weights(theta)
    in_maps = []
    for c in range(N_CORES):
        shard = np.ascontiguousarray(inputs[B_CORE * c:B_CORE * (c + 1)])
        in_maps.append({"xin": shard, "wstack": wstack})
    res = run_bass_kernel_spmd(nc, in_maps, core_ids=list(range(N_CORES)))
    outp = _out_perm()
    out = np.concatenate([r["out"][outp] for r in res.results], axis=0)
    return out.astype(np.float32)


# revision 3
# speedup vs baseline: 1.0060x; 1.0060x over previous
"""Trainium2 Bass kernel v2 for nn_EnhancedQuantumLayer.

Data parallel over 8 cores (64 samples each); per core two phase-shifted
half-batch pipelines of 32 samples ([128, 256]-f32 tiles).

Key points vs v1:
  - State is complex-interleaved bf16 (re, im) pairs viewed as one f32
    element, so each layout flip is ONE StreamTranspose per half.
  - Product state X built by PE matmuls (u-chunked, runtime stationaries
    from the doubling output) -- no DRAM bounce / repack DMAs.
  - Feature-map recursion runs on Pool (gpsimd) as a single-engine chain.
  - PE warm-up dummy matmuls ramp the tensor clock to 2.4 GHz before the
    real matmul stream starts.
  - Output gathered by 3 strided DMAs directly from SBUF.

Sample index b (0..63) = (h:1 | s1:1 | s0:1 | u:3).
Layout A: p = 64*s1 + 32*q4 + 16*s0 + h4 ; f = 32*u + l5
Layout B: p = 64*s1 + 32*q4 + l5        ; f = 32*u + 16*s0 + h4
"""

import numpy as np

N_QUBITS = 10
N_LAYERS = 4
FREQS = (1.0, 2.0, 4.0, 8.0, 16.0)
PI = float(np.pi)
B_TOTAL = 512
B_CORE = 64
N_CORES = 8
N_WARM = 12  # PE warm-up dummy matmuls

CZCNOT = np.array([[1, 0, 0, 0],
                   [0, 1, 0, 0],
                   [0, 0, 0, -1],
                   [0, 0, 1, 0]], dtype=np.complex128)


# ---------------------------------------------------------------- host math
def _rz(phi):
    return np.array([[np.exp(-0.5j * phi), 0], [0, np.exp(0.5j * phi)]],
                    dtype=np.complex128)


def _rx(th):
    c, s = np.cos(th / 2), np.sin(th / 2)
    return np.array([[c, -1j * s], [-1j * s, c]], dtype=np.complex128)


def _ry(th):
    c, s = np.cos(th / 2), np.sin(th / 2)
    return np.array([[c, -s], [s, c]], dtype=np.complex128)


def _kron_list(ms):
    out = ms[0]
    for m in ms[1:]:
        out = np.kron(out, m)
    return out


def _embed_2q(space_qubits, qa, qb, M4):
    n = len(space_qubits)
    dim = 2 ** n
    pa, pb = space_qubits.index(qa), space_qubits.index(qb)
    out = np.zeros((dim, dim), dtype=np.complex128)
    for idx in range(dim):
        bits = [(idx >> (n - 1 - i)) & 1 for i in range(n)]
        col4 = 2 * bits[pa] + bits[pb]
        for row4 in range(4):
            val = M4[row4, col4]
            if val != 0:
                nb = bits.copy()
                nb[pa], nb[pb] = row4 >> 1, row4 & 1
                ridx = sum(bit << (n - 1 - i) for i, bit in enumerate(nb))
                out[ridx, idx] += val
    return out


A6 = [4, "s0", 0, 1, 2, 3]
L6 = [4, 5, 6, 7, 8, 9]
_E_evenA6 = _embed_2q(A6, 0, 1, CZCNOT) @ _embed_2q(A6, 2, 3, CZCNOT)
_E_oddA6 = _embed_2q(A6, 3, 4, CZCNOT) @ _embed_2q(A6, 1, 2, CZCNOT)
_PermA6 = _E_oddA6 @ _E_evenA6
_E_evenL6 = _embed_2q(L6, 6, 7, CZCNOT) @ _embed_2q(L6, 8, 9, CZCNOT)
_E_oddL6 = _embed_2q(L6, 7, 8, CZCNOT) @ _embed_2q(L6, 5, 6, CZCNOT)
_CG64 = _embed_2q(L6, 4, 5, CZCNOT)


def _layer_matrices6(theta):
    ang = np.tanh(theta.astype(np.float64)) * PI
    S_A, S_L = [], []
    for layer in range(N_LAYERS):
        U = []
        for q in range(10):
            a0, a1, a2 = ang[layer, q]
            U.append(_rx(a0 * 0.5) @ _rz(a2) @ _ry(a1) @ _rz(a0))
        UA6 = _kron_list([U[4], np.eye(2), U[0], U[1], U[2], U[3]])
        UL6 = _kron_list([np.eye(2), U[5], U[6], U[7], U[8], U[9]])
        S_A.append(UA6 if layer == 0 else UA6 @ _PermA6)
        S_L.append(_E_oddL6 @ _E_evenL6 @ _CG64 @ UL6)
    return S_A, S_L


def _measurement_weights6():
    W1 = np.zeros((128, 32), dtype=np.float32)
    for s1 in range(2):
        for q4 in range(2):
            for l in range(32):
                p = 64 * s1 + 32 * q4 + l
                W1[p, 16 * s1 + 0] = 1.0 - 2.0 * q4
                for j in range(5):
                    W1[p, 16 * s1 + 1 + j] = 1.0 - 2.0 * ((l >> (4 - j)) & 1)
                W1[p, 16 * s1 + 6] = 1.0
    W2 = np.zeros((32, 16), dtype=np.float32)
    for s0 in range(2):
        for h4 in range(16):
            p = 16 * s0 + h4
            s = [1.0 - 2.0 * ((h4 >> (3 - i)) & 1) for i in range(4)]
            W2[p, 4 * s0 + 0] = s[0]
            W2[p, 4 * s0 + 1] = s[1] * s[0]
            W2[p, 4 * s0 + 2] = s[2] * s[1] * s[0]
            W2[p, 4 * s0 + 3] = s[3] * s[2]
            W2[p, 8 + s0] = 1.0
            W2[p, 10 + s0] = s[3] * s[2]
    return W1, W2


def _host_weights(theta):
    """wstack [128, 24*128] bf16-as-uint16: wstack[k, 128*m + c] = S_m[c, k].
    Order per layer: [Ar, Ain(-Ai), Ai, Lr, Lin, Li]."""
    S_A, S_L = _layer_matrices6(theta)
    I2 = np.eye(2)
    w = np.zeros((128, 24 * 128), np.float32)
    m = 0
    for layer in range(N_LAYERS):
        for S in [S_A[layer], S_L[layer]]:
            full = np.kron(I2, S)
            for comp in (full.real, -full.imag, full.imag):
                w[:, 128 * m:128 * m + 128] = comp.T.astype(np.float32)
                m += 1
    # f32 -> bf16 (round-to-nearest-even) stored as uint16
    u = w.view(np.uint32)
    rounded = ((u + 0x7FFF + ((u >> 16) & 1)) >> 16).astype(np.uint16)
    return np.ascontiguousarray(rounded)


def _f32_to_bf16_u16(a):
    a = np.ascontiguousarray(a.astype(np.float32))
    u = a.view(np.uint32)
    return (((u + 0x7FFF + ((u >> 16) & 1)) >> 16)).astype(np.uint16)


# ------------------------------------------------------------- bass builder
_BUILD_CACHE = {}


def _build_module():
    import concourse.bass as bass  # noqa
    import concourse.mybir as mybir
    from concourse import bacc
    from concourse.tile import TileContext

    f32 = mybir.dt.float32
    bf16 = mybir.dt.bfloat16
    AF = mybir.ActivationFunctionType
    OP = mybir.AluOpType

    nc = bacc.Bacc("TRN2", target_bir_lowering=False, debug=False)

    xin = nc.dram_tensor("xin", [B_CORE, 10], f32, kind="ExternalInput").ap()
    # bf16 payload shipped as uint16 (run_bass_kernel_spmd feeds numpy arrays)
    wstack = nc.dram_tensor("wstack", [128, 24 * 128], mybir.dt.uint16,
                            kind="ExternalInput").ap()
    out_d = nc.dram_tensor("out", [B_CORE, 10], f32, kind="ExternalOutput").ap()

    # ---- constants
    # trig planes: rows j=0..5 sin(c_j x), rows 6..11 cos(c_j x), c_j=0.25*2^j
    cs_mult = np.zeros((12, 10), dtype=np.float32)
    cs_bias = np.zeros((12, 10), dtype=np.float32)
    for j in range(6):
        cs_mult[j] = 0.25 * 2 ** j
        cs_mult[6 + j] = 0.25 * 2 ** j
        cs_bias[6 + j] = 0.5 * PI
    mult_np = np.tile(cs_mult.reshape(1, 120), (64, 1)).astype(np.float32)
    bias_np = np.tile(cs_bias.reshape(1, 120), (64, 1)).astype(np.float32)
    v0_np = np.zeros((64, 40), dtype=np.float32)
    v0_np[:, 0::4] = 1.0
    cpack = np.zeros((64, 280), dtype=np.float32)
    cpack[:, 0:120] = mult_np
    cpack[:, 120:240] = bias_np
    cpack[:, 240:280] = v0_np
    cpack_c = nc.inline_tensor(cpack, name="cpack").ap()

    W1_np, W2_np = _measurement_weights6()
    mask_np = np.zeros((64, 128), np.float32)
    for p in range(64):
        s1, s0 = (p >> 4) & 1, (p >> 3) & 1
        for q4 in range(2):
            c0 = 64 * s1 + 32 * q4 + 16 * s0
            mask_np[p, c0:c0 + 16] = 1.0
    mcmask_np = np.zeros((64, 512), np.float32)
    for p in range(64):
        u = p & 7
        mcmask_np[p, 64 * u:64 * u + 64] = 1.0
    wpack = np.zeros((128, 816), np.uint16)
    wpack[:, 0:32] = _f32_to_bf16_u16(W1_np)
    wpack[0:32, 32:48] = _f32_to_bf16_u16(W2_np)
    wpack[0:64, 48:176] = _f32_to_bf16_u16(mask_np)
    wpack[0:64, 176:304] = _f32_to_bf16_u16(-mask_np)
    wpack[0:64, 304:816] = _f32_to_bf16_u16(mcmask_np)
    wpack_c = nc.inline_tensor(wpack, name="wpack").ap()

    with TileContext(nc) as tc:
        with (
            tc.tile_pool(name="wpool", bufs=1) as wpool,
            tc.tile_pool(name="st", bufs=1) as st,      # state tiles
            tc.tile_pool(name="sm", bufs=2) as sm,      # small tiles
            tc.tile_pool(name="ps", bufs=1, space="PSUM") as ps,
        )            :
            # ---------------- phase 0: DMAs + PE warm-up
            # partition p = sample b = 32h + 16s1 + 8s0 + u
            sx = sm.tile([64, 10], f32, tag="sx")
            nc.sync.dma_start(sx[:], xin)
            ct = wpool.tile([64, 280], f32, tag="cp")
            nc.sync.dma_start(ct[:], cpack_c)
            wt = wpool.tile([128, 24 * 128], bf16, tag="w")
            nc.sync.dma_start(wt[:].bitcast(mybir.dt.uint16), wstack)
            mult_v = ct[:, 0:120]
            bias_v = ct[:, 120:240]
            v0_v = ct[:, 240:280]
            wq = wpool.tile([128, 816], bf16, tag="wq")
            w1_t = wq[:, 0:32]
            w2_t = wq[0:32, 32:48]
            mask_v = wq[0:64, 48:176]
            nmask_v = wq[0:64, 176:304]
            mcmask_v = wq[0:64, 304:816]

            junk = wpool.tile([128, 512], bf16, tag="junk")
            nc.gpsimd.memset(junk[:], 0.125)
            warm = ps.tile([128, 512], f32, tag="y0")
            for k in range(N_WARM):
                nc.tensor.matmul(warm[:], junk[:, 0:128], junk[:],
                                 start=True, stop=True)

            # ---------------- phase 1-2: tanh + trig table
            xt = sm.tile([64, 10], f32, tag="xt")
            nc.scalar.activation(xt[:], sx[:], AF.Tanh)
            nc.scalar.dma_start(wq[:].bitcast(mybir.dt.uint16), wpack_c)

            xb12 = (xt[:].unsqueeze(1).broadcast_to((64, 12, 10)))
            ma = sm.tile([64, 120], f32, tag="ma")
            nc.vector.tensor_tensor(
                ma[:].rearrange("p (r q) -> p r q", q=10), xb12,
                mult_v.rearrange("p (r q) -> p r q", q=10), OP.mult)
            nc.vector.tensor_tensor(ma[:], ma[:], bias_v, OP.add)
            MAGIC = 1.5 * 2 ** 23
            kk = sm.tile([64, 120], f32, tag="kk")
            nc.vector.tensor_scalar(kk[:], ma[:], 1.0 / (2.0 * PI), MAGIC,
                                    OP.mult, OP.add)
            nc.vector.tensor_scalar(kk[:], kk[:], MAGIC, None, OP.subtract)
            nc.vector.scalar_tensor_tensor(ma[:], kk[:], -2.0 * PI, ma[:],
                                           OP.mult, OP.add)
            PCLAMP = PI * (1.0 - 1e-6)
            nc.vector.tensor_scalar(ma[:], ma[:], PCLAMP, -PCLAMP,
                                    OP.min, OP.max)

            # tb rows: s (0:60) | ns (60:120) | s2 (120:180) | cos (180:240)
            tb = sm.tile([64, 240], f32, tag="tb")
            # chunk A: freqs 0-2 ; chunk B: freqs 3-5
            for lo, hi in ((0, 30), (30, 60)):
                nc.scalar.activation(tb[:, lo:hi], ma[:, lo:hi], AF.Sin)
                # cos rows
                nc.scalar.activation(tb[:, 180 + lo:180 + hi],
                                     ma[:, 60 + lo:60 + hi], AF.Sin)
                # ns = -s ; s2 = s
                nc.gpsimd.tensor_scalar(tb[:, 60 + lo:60 + hi],
                                        tb[:, lo:hi], -1.0, None, OP.mult)
                nc.gpsimd.tensor_copy(tb[:, 120 + lo:120 + hi], tb[:, lo:hi])

            tb_v = tb[:].rearrange("p (r q) -> p r q", q=10)  # [64, 24, 10]

            # ---------------- phase 3: v-recursion on Pool
            # closed form for steps 0-1: v = RX(x/2) RZ(x) |0>:
            # alpha = cos(x/4)*(cos x/2, -sin x/2); beta = -sin(x/4)*(sin
            # x/2, cos x/2).  tb rows: sin j=1 -> 1, ns j=0 -> 6, ns j=1
            # -> 7, cos j=0 -> 18, cos j=1 -> 19.
            vinit = sm.tile([64, 40], f32, tag="va")
            vi_v = vinit[:].rearrange("p (q t c) -> p q t c", t=2, c=2)
            cx4 = (tb_v[:, 18:19, :].transpose([0, 2, 1]).unsqueeze(2)
                   .broadcast_to((64, 10, 1, 2)))
            zpair = (tb_v[:, 19:6:-12, :].transpose([0, 2, 1]).unsqueeze(2))
            nsx4 = (tb_v[:, 6:7, :].transpose([0, 2, 1]).unsqueeze(2)
                    .broadcast_to((64, 10, 1, 2)))
            spair = (tb_v[:, 1:20:18, :].transpose([0, 2, 1]).unsqueeze(2))
            nc.gpsimd.tensor_tensor(vi_v[:, :, 0:1, :], cx4, zpair, OP.mult)
            nc.gpsimd.tensor_tensor(vi_v[:, :, 1:2, :], nsx4, spair, OP.mult)
            v_cur = vinit
            for k in range(2, 10):
                is_rz = (k % 2 == 0)
                lv = k // 2 + 1 if is_rz else k // 2
                cplane = (tb_v[:, 18 + lv, :].unsqueeze(2)
                          .broadcast_to((64, 10, 4))
                          .rearrange("p q (a b) -> p q a b", a=2))
                t1 = sm.tile([64, 40], f32, tag="t1")
                t2 = sm.tile([64, 40], f32, tag="t2")
                t1v = t1[:].rearrange("p (q a b) -> p q a b", a=2, b=2)
                t2v = t2[:].rearrange("p (q a b) -> p q a b", a=2, b=2)
                vv = v_cur[:].rearrange("p (q a b) -> p q a b", a=2, b=2)
                nc.gpsimd.tensor_tensor(t1v, vv, cplane, OP.mult)
                spq = tb[:].rearrange("p (r q) -> p q r", q=10)  # [64,10,24]
                if is_rz:
                    # t2[t, c] = vpart[t, c] * row(lv + 6*(t+c)); (s, ns, s2)
                    vpart = vv[:, :, :, ::-1]
                    s_t0 = spq[:, :, lv:lv + 7:6].unsqueeze(2)    # [64,10,1,2]
                    s_t1 = spq[:, :, lv + 6:lv + 13:6].unsqueeze(2)
                    nc.gpsimd.tensor_tensor(t2v[:, :, 0:1, :],
                                            vpart[:, :, 0:1, :], s_t0, OP.mult)
                    nc.gpsimd.tensor_tensor(t2v[:, :, 1:2, :],
                                            vpart[:, :, 1:2, :], s_t1, OP.mult)
                else:
                    vpart = vv[:, :, ::-1, ::-1]
                    sview = (spq[:, :, lv:lv + 7:6].unsqueeze(2)
                             .broadcast_to((64, 10, 2, 2)))
                    nc.gpsimd.tensor_tensor(t2v, vpart, sview, OP.mult)
                v_nxt = sm.tile([64, 40], f32, tag="vb" if k % 2 == 0 else "va")
                nc.gpsimd.tensor_tensor(v_nxt[:], t1[:], t2[:], OP.add)
                v_cur = v_nxt

            # ---------------- phase 4: H/L doubling (Pool), G [64,64] r/i
            H_QUBITS = [4, 0, 1, 2, 3]
            L_QUBITS = [5, 6, 7, 8, 9]
            g_ar = sm.tile([64, 64], f32, tag="gra")
            g_ai = sm.tile([64, 64], f32, tag="gia")
            g_br = sm.tile([64, 64], f32, tag="grb")
            g_bi = sm.tile([64, 64], f32, tag="gib")
            g_r, g_i = g_ar, g_ai
            vvq = v_cur[:].rearrange("p (q t c) -> p q t c", t=2, c=2)
            g_r0 = g_r[:].rearrange("p (s x) -> p s x", s=2)[:, :, 0:2]
            g_i0 = g_i[:].rearrange("p (s x) -> p s x", s=2)[:, :, 0:2]
            nc.gpsimd.tensor_copy(g_r0, vvq[:, 4:6, :, 0])
            nc.gpsimd.tensor_copy(g_i0, vvq[:, 4:6, :, 1])
            ptA_full = sm.tile([64, 128], f32, tag="ptA")
            ptB_full = sm.tile([64, 128], f32, tag="ptB")
            for j in range(1, 5):
                w = 2 ** j
                qH = H_QUBITS[j]
                ptA = ptA_full[:, 0:8 * w]
                ptB = ptB_full[:, 0:8 * w]
                gr_b = (g_r[:].rearrange("p (s x) -> p s x", s=2)[:, :, 0:w]
                        .unsqueeze(3).broadcast_to((64, 2, w, 2)))
                gi_b = (g_i[:].rearrange("p (s x) -> p s x", s=2)[:, :, 0:w]
                        .unsqueeze(3).broadcast_to((64, 2, w, 2)))
                vsel = vvq[:, qH:qH + 7:6]          # [64, 2q, 2t, 2c]
                vA = (vsel.transpose([0, 3, 1, 2])
                      .unsqueeze(3).broadcast_to((64, 2, 2, w, 2)))
                vB = (vsel[:, :, :, ::-1].transpose([0, 3, 1, 2])
                      .unsqueeze(3).broadcast_to((64, 2, 2, w, 2)))
                ptA_v = ptA.rearrange("p (c s x t) -> p c s x t",
                                      c=2, s=2, t=2)
                ptB_v = ptB.rearrange("p (c s x t) -> p c s x t",
                                      c=2, s=2, t=2)
                for c in range(2):
                    nc.gpsimd.tensor_tensor(ptA_v[:, c], gr_b, vA[:, c],
                                            OP.mult)
                    nc.gpsimd.tensor_tensor(ptB_v[:, c], gi_b, vB[:, c],
                                            OP.mult)
                g2_r, g2_i = (g_br, g_bi) if j % 2 else (g_ar, g_ai)
                g2r_v = g2_r[:].rearrange("p (s h t) -> p s h t",
                                          s=2, t=2)[:, :, 0:w, :]
                g2i_v = g2_i[:].rearrange("p (s h t) -> p s h t",
                                          s=2, t=2)[:, :, 0:w, :]
                nc.gpsimd.tensor_tensor(g2r_v, ptA_v[:, 0], ptB_v[:, 0],
                                        OP.subtract)
                nc.gpsimd.tensor_tensor(g2i_v, ptA_v[:, 1], ptB_v[:, 1],
                                        OP.add)
                g_r, g_i = g2_r, g2_i

            # ---------------- phase 5: S_H assembly + M_c build
            # sh_* [64, 128] bf16: row p; cols = 64s1' + 32q4 + 16s0' + h4,
            # = G_H[p, (q4, h4)] * mask(s1'=s1(p), s0'=s0(p))
            sh_r = st.tile([64, 128], bf16, tag="shr")
            sh_i = st.tile([64, 128], bf16, tag="shi")
            sh_in = st.tile([64, 128], bf16, tag="shin")

            def bc(gt):
                return (gt[:, 0:32]
                        .rearrange("p (q4 h4) -> p q4 h4", q4=2)
                        .unsqueeze(2)
                        .broadcast_to((64, 2, 2, 16)))

            def shv(sh, a):
                return (sh[:, 64 * a:64 * a + 64]
                        .rearrange("p (q4 b h4) -> p q4 b h4", q4=2, b=2))

            def mkv(mk, a):
                return (mk[:, 64 * a:64 * a + 64]
                        .rearrange("p (q4 b h4) -> p q4 b h4", q4=2, b=2))

            for a in range(2):
                nc.gpsimd.tensor_tensor(shv(sh_r, a), bc(g_r),
                                        mkv(mask_v, a), OP.mult)
                nc.vector.tensor_tensor(shv(sh_i, a), bc(g_i),
                                        mkv(mask_v, a), OP.mult)
                nc.gpsimd.tensor_tensor(shv(sh_in, a), bc(g_i),
                                        mkv(nmask_v, a), OP.mult)
            # gl_c [64, 64] bf16 interleaved (Gr, Gi) pairs
            gl_c = st.tile([64, 64], bf16, tag="glc")
            glv = gl_c[:].rearrange("p (l c) -> p l c", c=2)
            nc.vector.tensor_copy(glv[:, :, 0], g_r[:, 32:64])
            nc.vector.tensor_copy(glv[:, :, 1], g_i[:, 32:64])
            # M_c [64, 512] bf16: row p -> cols 64u(p)..+64 = gl_c row p
            # dense broadcast * mask (diagonal not AP-expressible)
            m_c = st.tile([64, 512], bf16, tag="mc")
            nc.vector.tensor_tensor(
                m_c[:].rearrange("p (v x) -> p v x", v=8),
                gl_c[:].unsqueeze(1).broadcast_to((64, 8, 64)),
                mcmask_v.rearrange("p (v x) -> p v x", v=8), OP.mult)

            # ---------------- phase 6: X-build MMs (4 per half)
            def ri(bank):
                return bank[:, 0:256], bank[:, 256:512]

            def evict1(dst_f32tile, bank, h=0):
                # one op: [p, (c,f)] psum -> interleaved bf16 [p, (f,c)]
                dst = (dst_f32tile[:].bitcast(bf16)
                       .rearrange("p (f c) -> p c f", c=2))
                srcv = bank[:].rearrange("p (c f) -> p c f", c=2)
                nc.scalar.activation(dst, srcv, AF.Copy)

            x_bank = [ps.tile([128, 512], f32, tag=f"x{h}", name=f"xb{h}")
                      for h in range(2)]
            x_ps = [ri(t) for t in x_bank]
            for h in range(2):
                xr, xi = x_ps[h]
                rows = slice(32 * h, 32 * h + 32)
                mv = m_c[rows, :].rearrange("p (f c) -> p f c", c=2)
                m_ev, m_od = mv[:, :, 0], mv[:, :, 1]
                nc.tensor.matmul(xr, sh_r[rows, :], m_ev,
                                 start=True, stop=False)
                nc.tensor.matmul(xr, sh_in[rows, :], m_od,
                                 start=False, stop=True)
                nc.tensor.matmul(xi, sh_r[rows, :], m_od,
                                 start=True, stop=False)
                nc.tensor.matmul(xi, sh_i[rows, :], m_ev,
                                 start=False, stop=True)

            # ---------------- phase 7: layers
            def W(m):
                return wt[:, 128 * m:128 * m + 128]

            def evict(dst_f32tile, src_ps_r, src_ps_i, eng_r, eng_i):
                dv = dst_f32tile[:].bitcast(bf16).rearrange(
                    "p (f c) -> p f c", c=2)
                for eng, view, srcp in ((eng_r, dv[:, :, 0], src_ps_r),
                                        (eng_i, dv[:, :, 1], src_ps_i)):
                    if eng is nc.scalar:
                        nc.scalar.activation(view, srcp, AF.Copy)
                    else:
                        eng.tensor_copy(view, srcp)

            xa = [st.tile([128, 256], f32, tag=f"xa{h}", name=f"xa{h}")
                  for h in range(2)]
            xb = [st.tile([128, 256], f32, tag=f"xb{h}", name=f"xb{h}")
                  for h in range(2)]
            for h in range(2):
                evict1(xa[h], x_bank[h], h)

            def re_im(t):
                v = t[:].bitcast(bf16).rearrange("p (f c) -> p f c", c=2)
                return v[:, :, 0], v[:, :, 1]

            z_ps = [None, None]
            z_bank = [None, None]
            for layer in range(N_LAYERS):
                base = 6 * layer
                y_bank = [ps.tile([128, 512], f32, tag=f"y{h}",
                                  name=f"ybank{h}") for h in range(2)]
                y_ps = [ri(t) for t in y_bank]
                # A-side MMs (both halves)
                for h in range(2):
                    yr, yi = y_ps[h]
                    xre, xim = re_im(xa[h])
                    nc.tensor.matmul(yr, W(base + 0), xre,
                                     start=True, stop=False)
                    nc.tensor.matmul(yr, W(base + 1), xim,
                                     start=False, stop=True)
                    nc.tensor.matmul(yi, W(base + 0), xim,
                                     start=True, stop=False)
                    nc.tensor.matmul(yi, W(base + 2), xre,
                                     start=False, stop=True)
                # one full-bank evict (ACT) + one combined flip (DVE)
                for h in range(2):
                    yb = st.tile([128, 256], f32, tag=f"yb{h}")
                    evict1(yb, y_bank[h], h)
                    nc.vector.transpose(xb[h][:], yb[:])
                # L-side MMs
                for h in range(2):
                    zb_t = ps.tile([128, 512], f32, tag=f"z{h}",
                                   name=f"zbank{h}")
                    z_bank[h] = zb_t
                    z_ps[h] = ri(zb_t)
                    zr, zi = z_ps[h]
                    bre, bim = re_im(xb[h])
                    nc.tensor.matmul(zr, W(base + 3), bre,
                                     start=True, stop=False)
                    nc.tensor.matmul(zr, W(base + 4), bim,
                                     start=False, stop=True)
                    nc.tensor.matmul(zi, W(base + 3), bim,
                                     start=True, stop=False)
                    nc.tensor.matmul(zi, W(base + 5), bre,
                                     start=False, stop=True)
                if layer < N_LAYERS - 1:
                    for h in range(2):
                        zb = st.tile([128, 256], f32, tag=f"zb{h}")
                        evict1(zb, z_bank[h], h)
                        nc.vector.transpose(xa[h][:], zb[:])

            # ---------------- phase 8: measurement (layout B, per half)
            res = st.tile([32, 512], f32, tag="res")
            nc.gpsimd.memset(res[:], 0.0)
            res_t = st.tile([32, 512], f32, tag="rest")
            o_bank = ps.tile([32, 512], f32, tag="obank")
            o2_bank = ps.tile([16, 512], f32, tag="o2bank")
            for h in range(2):
                sq = st.tile([128, 512], f32, tag=f"sq{h}")
                nc.scalar.square(sq[:], z_bank[h][:])
                p_bf = st.tile([128, 256], bf16, tag=f"pbf{h}")
                nc.gpsimd.tensor_tensor(p_bf[:], sq[:, 0:256],
                                        sq[:, 256:512], OP.add)
                o1 = o_bank[:, 256 * h:256 * h + 256]
                nc.tensor.matmul(o1, w1_t, p_bf[:], start=True, stop=True)
                o1t = st.tile([32, 256], f32, tag=f"o1t{h}")
                nc.vector.transpose(o1t[:], o1)
                o1b = st.tile([32, 256], bf16, tag=f"o1b{h}")
                nc.gpsimd.tensor_copy(o1b[:], o1t[:])
                o2 = o2_bank[:, 256 * h:256 * h + 256]
                nc.tensor.matmul(o2, w2_t, o1b[:], start=True, stop=True)
                nc.scalar.activation(res[0:16, 256 * h:256 * h + 256],
                                     o2, AF.Copy)
                nc.vector.transpose(res_t[:, 256 * h:256 * h + 256],
                                    res[:, 256 * h:256 * h + 256])

            # ---------------- phase 9: gather
            # res[j2, 256h + 32u + 16s1 + j], j2 = 4 s0 + t (t<4) | 8+s0
            # out row r = 32h + 16 s0 + 2u + s1 (host un-permutes)
            # res_t[16 s1 + j, 256h + 32u + j2]
            out_v = out_d.rearrange("(h s0 u s1) q -> h s0 u s1 q",
                                    h=2, s0=2, u=8)
            res_v2 = res[:].rearrange("p (h u s1 j) -> p h u s1 j",
                                      h=2, u=8, s1=2)
            rtv = res_t[:].rearrange("p (h u f) -> p h u f", h=2, f=32)
            # piece 1 (q0-3): per (s1, s0): src row 16s1+6: (h, u, t)
            for s1 in range(2):
                for s0 in range(2):
                    eng = nc.sync if s0 == 0 else nc.scalar
                    eng.dma_start(
                        out_v[:, s0, :, s1, 0:4].opt(),
                        rtv[16 * s1 + 6:16 * s1 + 7, :, :,
                            4 * s0:4 * s0 + 4].opt())
            for h in range(2):
                # piece 2 (q4): rows 10:12 (dup W2 cols), j=0
                nc.gpsimd.dma_start(
                    out_v[h, :, :, :, 4:5],
                    res_v2[10:12, h, :, :, 0:1])
                # piece 3 (q5-9): rows 8:10
                eng = nc.sync if h == 0 else nc.scalar
                eng.dma_start(
                    out_v[h, :, :, :, 5:10],
                    res_v2[8:10, h, :, :, 1:6])

    import concourse.bacc as _bacc
    _orig_tables = _bacc.get_activation_tables

    def _patched_tables(arch):
        t = dict(_orig_tables(arch))
        AFt = mybir.ActivationFunctionType
        strip = {AFt.Tanh, AFt.Sin, AFt.Square, AFt.Copy}
        out = {}
        for name, s in t.items():
            out[name] = s if name == "silu_and_others" else (s - strip)
        return out

    _bacc.get_activation_tables = _patched_tables
    try:
        nc.finalize()
    finally:
        _bacc.get_activation_tables = _orig_tables
    # the pass emits an unconditional set-0 load at entry plus the real
    # set load before the first activation; merge them into one.
    for blk in nc.m.functions[0].blocks:
        insts = blk.instructions
        loads = [(i, x) for i, x in enumerate(insts)
                 if isinstance(x, mybir.InstLoadActFuncSet)]
        if (len(loads) == 2 and not any(x.has_wait() or x.has_update()
                                        for _, x in loads)):
            loads[0][1].act_func_set_id = loads[1][1].act_func_set_id
            del insts[loads[1][0]]
    return nc


def _get_module():
    if "nc" not in _BUILD_CACHE:
        _BUILD_CACHE["nc"] = _build_module()
    return _BUILD_CACHE["nc"]


# ---------------------------------------------------------------- entrypoint
def _out_perm():
    """outp[b] = device-out row holding sample b's result.
    Device writes sample (h,s1,s0,u) to row r = 32h + 16s0 + 2u + s1."""
    outp = np.zeros(64, np.int64)
    for b in range(64):
        h, s1, s0, u = b >> 5, (b >> 4) & 1, (b >> 3) & 1, b & 7
        outp[b] = 32 * h + 16 * s0 + 2 * u + s1
    return outp


def kernel(inputs, theta):
    inputs = np.asarray(inputs, dtype=np.float32)
    theta = np.asarray(theta, dtype=np.float32)
    assert inputs.shape == (B_TOTAL, N_QUBITS)

    from concourse.bass_utils import run_bass_kernel_spmd

    nc = _get_module()
    wstack = _host_weights(theta)
    in_maps = []
    for c in range(N_CORES):
        shard = np.ascontiguousarray(inputs[B_CORE * c:B_CORE * (c + 1)])
        in_maps.append({"xin": shard, "wstack": wstack})
    res = run_bass_kernel_spmd(nc, in_maps, core_ids=list(range(N_CORES)))
    outp = _out_perm()
    out = np.concatenate([r["out"][outp] for r in res.results], axis=0)
    return out.astype(np.float32)
